# revision 1
# baseline (speedup 1.0000x reference)
"""Trainium2 Bass kernel for nn_Encoder (GNN message passing / SAT encoder).

Design v2 (8 NeuronCores, data-parallel over V):
  - Global pair-token table gtab[row(v)] = (emb_v bf16[64] || neg_emb_v bf16[64])
    (256B rows, 20481 rows < int16 range). Iter-1 table host-built; iter-2
    table produced by ONE AllGather of per-shard pair rows. No per-core
    compact-table rebuild.
  - Clause stage: transpose-mode dma_gather (16-bit xbar) returns literal
    tokens E-MAJOR: partition p = bf16 element p -> partitions 0:64 plain,
    64:128 negated. A single copy_predicated per call selects the negated
    half per literal. No PE transposes or PSUM->SBUF shuffles; matmuls are
    64-contraction bf16 with batched 4-tile moving dim.
  - l2norm is deferred: sum-of-squares via a ones-matmul (partition
    broadcast), then ONE Rsqrt activation per stage (no act-table thrash),
    then one big elementwise multiply.
  - Variable stage: row gather of c_emb (fp32, 256B rows) from DRAM into
    dense (v,c) layout, PE transposes, 128-contraction bf16 matmuls.
"""

import math
from contextlib import ExitStack

import numpy as np

import concourse.bass as bass
import concourse.bacc as bacc
import concourse.mybir as mybir
import concourse.tile as tile
from concourse.bass_utils import run_bass_kernel_spmd
from concourse.masks import make_identity

F32 = mybir.dt.float32
BF16 = mybir.dt.bfloat16
I16 = mybir.dt.int16
I8 = mybir.dt.int8
AF = mybir.ActivationFunctionType
ALU = mybir.AluOpType

NCORES = 8
P = 128


def _l2norm(x):
    n = np.sqrt(np.sum(x * x, axis=-1, keepdims=True))
    return x / np.maximum(n, 1e-12)


def _wrap_idx(flat):
    """dma_gather idx layout: idx j -> [j%16, j//16] int16, replicated x8."""
    n = len(flat)
    assert n % 16 == 0
    w = np.asarray(flat, np.int16).reshape(-1, 16).T.copy()
    return np.tile(w, (8, 1))


def _ceil_to(x, m):
    return ((x + m - 1) // m) * m


class HostPlan:
    """Static index math; identical program across cores (SPMD)."""

    def __init__(self, V, G, C, M, E, clause_lits, lit_neg, lit_valid,
                 clause_valid):
        self.V, self.G, self.C, self.M, self.E = V, G, C, M, E
        VS = V // NCORES
        self.VS = VS
        self.NG = _ceil_to(VS, P) // P
        self.VSP = self.NG * P

        # literal rotation (prepare_variables)
        var_ids = np.arange(V)[:, None, None]
        ind = np.argmax((clause_lits == var_ids) & (lit_valid > 0), axis=-1)
        p = np.arange(M)[None, None, :]
        ind_b = ind[:, :, None]
        gidx = np.where(p == 0, ind_b, np.where(p - 1 < ind_b, p - 1, p))
        self.psrc = np.take_along_axis(clause_lits, gidx, 2)   # [V,C,M]
        self.pneg = np.take_along_axis(lit_neg, gidx, 2) > 0
        self.pval = np.take_along_axis(lit_valid, gidx, 2) > 0
        self.cval = clause_valid > 0                           # [V,C]
        self.has_clause = self.cval.any(1)                     # [V]

        # global pair-table row of var v
        vv = np.arange(V)
        self.gtab_row = (vv // VS) * self.VSP + (vv % VS)
        self.GT_DATA = NCORES * self.VSP            # 20480
        self.FALSE_TOK = self.GT_DATA               # row 20480
        self.GT_ROWS = self.GT_DATA + 1
        assert self.GT_ROWS < 32767

        # per-core valid (v,c) rows, v-major
        self.rows = []
        for k in range(NCORES):
            vlo = k * VS
            rv, rc = np.nonzero(self.cval[vlo:vlo + VS])
            self.rows.append((len(rv), rv, rc))
        maxrows = max(r[0] for r in self.rows)
        self.NTILES = _ceil_to(_ceil_to(maxrows, P) // P, 4)
        self.NR = self.NTILES * P
        # split c_emb into two overlapping tile-range tensors so early x2
        # gathers depend only on the first half of the clause stage.
        # Bounds are data-driven: A must cover rows of x2 calls < X2_SPLIT,
        # B must cover rows of the remaining calls.
        self.X2_SPLIT = 2
        split_var = self.X2_SPLIT * 4 * P
        rmax_A = max(int(np.count_nonzero(r[1] < split_var))
                     for r in self.rows)
        rmin_B = min(int(np.searchsorted(r[1], split_var))
                     for r in self.rows)
        self.TA = min(_ceil_to(_ceil_to(rmax_A, P) // P + 1, 4), self.NTILES)
        self.TB0 = max((rmin_B // P - 1) // 4 * 4, 0)
        self.NTB = self.NTILES - self.TB0
        self.ZROW_A = self.TA
        self.ZROW_B = self.NTB
        self.CEMB_A_ROWS = P * (self.TA + 1)
        self.CEMB_B_ROWS = P * (self.NTB + 1)

        # per-core tensors
        self.idx_main = []   # [128, NR*M/16] i16  (stream (t, m, p))
        self.pred_neg = []   # [64, NR*M] int8
        self.idx_x2 = []     # [128, VSP*C/16] i16 (stream (g, c, p))
        self.noclE = []      # [64, VSP] int8 (1 = keep old value)
        for k in range(NCORES):
            vlo = k * VS
            nr, rv, rc = self.rows[k]
            src = self.psrc[vlo + rv, rc]          # [nr, M]
            neg = self.pneg[vlo + rv, rc]
            val = self.pval[vlo + rv, rc]
            tok = np.where(val, self.gtab_row[src], self.FALSE_TOK)
            full = np.full((self.NR, M), self.FALSE_TOK, np.int64)
            full[:nr] = tok
            pr = np.zeros((self.NR, M), np.int8)
            pr[:nr] = (val & neg)
            # stream order (t, m, p)
            st = full.reshape(self.NTILES, P, M).transpose(0, 2, 1).reshape(-1)
            sp = pr.reshape(self.NTILES, P, M).transpose(0, 2, 1).reshape(-1)
            self.idx_main.append(_wrap_idx(st))
            pred = np.concatenate([np.tile(sp[None, :], (64, 1)),
                                   np.tile(1 - sp[None, :], (64, 1))], axis=0)
            self.pred_neg.append(pred.copy())

            # x2: dense (v, c) -> cemb dram row, A/B-relative per call
            r = np.arange(nr)
            rowA = np.full((self.VSP, C), self.ZROW_A, np.int64)
            rowA[rv, rc] = (r % P) * (self.TA + 1) + r // P
            rowB = np.full((self.VSP, C), self.ZROW_B, np.int64)
            rowB[rv, rc] = (r % P) * (self.NTB + 1) + (r // P - self.TB0)
            split_var = self.X2_SPLIT * 4 * P
            assert np.count_nonzero(rv < split_var) <= self.TA * P
            assert np.searchsorted(rv, split_var) >= self.TB0 * P
            xa = rowA.reshape(self.NG, P, C).transpose(0, 2, 1)
            xb = rowB.reshape(self.NG, P, C).transpose(0, 2, 1)
            ns = self.X2_SPLIT * 4
            x2 = np.concatenate([xa[:ns].reshape(-1), xb[ns:].reshape(-1)])
            self.idx_x2.append(_wrap_idx(x2))

            hc = np.ones((self.VSP,), np.int8)
            hc[:VS] = (~self.has_clause[vlo:vlo + VS]).astype(np.int8)
            self.noclE.append(np.tile(hc[None, :], (64, 1)).copy())


def build_program(plan: HostPlan, TB=8, GB=4, stages=99):
    V, C, M, E = plan.V, plan.C, plan.M, plan.E
    NT, NG, VSP = plan.NTILES, plan.NG, plan.VSP
    NR = plan.NR
    nc = bacc.Bacc("TRN2", num_devices=NCORES,
                   dynamic_dma_scratch_size=16384)

    ein = lambda name, shape, dt=F32: nc.dram_tensor(
        name, shape, dt, kind="ExternalInput")
    t_gtab1 = ein("gtab1", [plan.GT_ROWS, 2 * E], BF16)
    t_idx_main = ein("idx_main", [P, NR * M // 16], I16)
    t_pred = ein("pred_neg", [P, NR * M], I8)
    t_idx_x2 = ein("idx_x2", [P, VSP * C // 16], I16)
    t_shard0 = ein("shard0E", [E, VSP])
    t_nocl = ein("noclE", [E, VSP], I8)
    t_wvc = ein("wvc", [P, M * P], BF16)
    t_wcc = ein("wcc", [P, (C // 2) * P], BF16)
    t_wneg = ein("wneg", [E, E], BF16)
    t_bvc = ein("bvc", [E, 1])
    t_bvc2 = ein("bvc2", [E, 1])
    t_bcc = ein("bcc", [E, 1])
    t_bcc2 = ein("bcc2", [E, 1])
    t_bneg = ein("bneg", [E, 1])
    t_false = ein("false_pair", [1, 2 * E], BF16)

    t_out = nc.dram_tensor("out_shardE", [E, VSP], F32, kind="ExternalOutput")

    t_gtab2 = nc.dram_tensor("gtab2", [plan.GT_ROWS, 2 * E], BF16,
                             addr_space="Shared")
    t_gtab2f = nc.dram_tensor("gtab2f", [plan.GT_ROWS, 2 * E], BF16,
                              kind="ExternalOutput")
    TA, TB0, NTB = plan.TA, plan.TB0, plan.NTB
    t_cembA = [nc.dram_tensor(f"cembA{i}", [plan.CEMB_A_ROWS, E], F32,
                              kind="ExternalOutput") for i in range(2)]
    t_cembB = [nc.dram_tensor(f"cembB{i}", [plan.CEMB_B_ROWS, E], F32,
                              kind="ExternalOutput") for i in range(2)]
    t_ag_in = nc.dram_tensor("ag_in", [VSP, 2 * E], BF16)
    t_warm_in = nc.dram_tensor("warm_in", [1, 2 * E], BF16)
    t_warm_out = nc.dram_tensor("warm_out", [NCORES, 2 * E], BF16)

    KC2 = (C * E) // P  # 4 contraction chunks for variable stage

    with tile.TileContext(nc) as tc, ExitStack() as ctx:
        const = ctx.enter_context(tc.tile_pool(name="const", bufs=1))
        sb_x = ctx.enter_context(tc.tile_pool(name="sb_x", bufs=3))
        sb_pred = ctx.enter_context(tc.tile_pool(name="sb_pred", bufs=2))
        sb_x2 = ctx.enter_context(tc.tile_pool(name="sb_x2", bufs=5))
        sb_ep = ctx.enter_context(tc.tile_pool(name="sb_ep", bufs=3))
        sb_u = ctx.enter_context(tc.tile_pool(name="sb_u", bufs=3))
        sb_xt = ctx.enter_context(tc.tile_pool(name="sb_xt", bufs=2))
        ps_mm = ctx.enter_context(tc.tile_pool(name="ps_mm", bufs=2, space="PSUM"))
        ps_s = ctx.enter_context(tc.tile_pool(name="ps_s", bufs=2, space="PSUM"))
        ps_tp = ctx.enter_context(tc.tile_pool(name="ps_tp", bufs=2, space="PSUM"))

        # ---- constants ----
        ident = const.tile([P, P], F32)
        make_identity(nc, ident)
        ident_bf = const.tile([P, P], BF16)
        nc.vector.tensor_copy(ident_bf[:], ident[:])
        ones = const.tile([E, E], F32)
        nc.vector.memset(ones[:], 1.0)
        wvc = const.tile([P, M * P], BF16)
        nc.sync.dma_start(wvc[:], t_wvc[:])
        wcc = const.tile([P, KC2 * P], BF16)
        nc.sync.dma_start(wcc[:], t_wcc[:])
        wneg = const.tile([E, E], BF16)
        nc.sync.dma_start(wneg[:], t_wneg[:])
        bvc = const.tile([E, 1], F32)
        nc.sync.dma_start(bvc[:], t_bvc[:])
        bvc2 = const.tile([E, 1], F32)
        nc.sync.dma_start(bvc2[:], t_bvc2[:])
        bcc = const.tile([E, 1], F32)
        nc.sync.dma_start(bcc[:], t_bcc[:])
        bcc2 = const.tile([E, 1], F32)
        nc.sync.dma_start(bcc2[:], t_bcc2[:])
        bneg = const.tile([E, 1], F32)
        nc.sync.dma_start(bneg[:], t_bneg[:])
        idxm = const.tile([P, NR * M // 16], I16)
        nc.sync.dma_start(idxm[:], t_idx_main[:])
        idxx2 = const.tile([P, VSP * C // 16], I16)
        nc.sync.dma_start(idxx2[:], t_idx_x2[:])
        nocl = const.tile([E, VSP], I8)
        nc.sync.dma_start(nocl[:], t_nocl[:])
        false_sb = const.tile([1, 2 * E], BF16)
        nc.sync.dma_start(false_sb[:], t_false[:])
        zero64 = const.tile([P, E], F32)
        nc.vector.memset(zero64[:], 0.0)
        epsv = const.tile([E, 1], F32)
        nc.vector.memset(epsv[:], 1e-24)
        zsel = const.tile([P, 1], BF16)
        nc.vector.memset(zsel[:], 0.0)

        shardE = [const.tile([E, VSP], F32, tag=f"shE{i}", name=f"shE{i}")
                  for i in range(2)]
        nc.sync.dma_start(shardE[0][:], t_shard0[:])

        # startup: zero rows of cemb tensors + false row of gtab2
        for i in range(2):
            cva = t_cembA[i][:].rearrange("(p t) e -> p t e", t=TA + 1)
            nc.sync.dma_start(cva[:, TA:TA + 1, :],
                              zero64[:].rearrange("p (o e) -> p o e", o=1))
            cvb = t_cembB[i][:].rearrange("(p t) e -> p t e", t=NTB + 1)
            nc.sync.dma_start(cvb[:, NTB:NTB + 1, :],
                              zero64[:].rearrange("p (o e) -> p o e", o=1))
        nc.sync.dma_start(t_gtab2f[plan.FALSE_TOK:plan.FALSE_TOK + 1, :],
                          false_sb[:])
        nc.sync.dma_start(t_warm_in[:], false_sb[:])
        nc.gpsimd.collective_compute(
            "AllGather", ALU.bypass, replica_groups=[list(range(NCORES))],
            ins=[t_warm_in[:]], outs=[t_warm_out[:]])

        # persistent big buffers
        newvE_bf = const.tile([E, VSP], BF16)
        negE = const.tile([E, VSP], BF16)
        pairbuf = const.tile([P, NG * 2 * E], BF16)

        n_iters = 1 if stages < 3 else 2
        ncalls = math.ceil(NT / TB)
        ncalls2 = math.ceil(NG / GB)
        NPRE = 3 if stages >= 5 else 0
        sem_ctr = [0]

        def new_sem():
            sem_ctr[0] += 1
            return nc.alloc_semaphore(f"swdge{sem_ctr[0]}")

        def issue_clause_gather(it, call, prepare):
            ctab = t_gtab1 if it == 0 else t_gtab2f
            t0 = call * TB
            tb = min(TB, NT - t0)
            n = tb * M * P
            col0 = t0 * M * P
            X = sb_x.tile([P, TB * M * P], BF16, tag="X")
            kw = {}
            if prepare:
                kw = dict(prepare_only=True, sem=new_sem())
            nc.gpsimd.dma_gather(
                X[:, :n].rearrange("p (o n) -> p o n", o=1),
                ctab[:], idxm[:, col0 // 16:(col0 + n) // 16], n, n,
                2 * E, transpose=True, single_packet=False, **kw)
            pred = sb_pred.tile([P, TB * M * P], I8, tag="pred")
            nc.sync.dma_start(pred[:, :n], t_pred[:, col0:col0 + n])
            return X, pred

        def consume_clause_call(it, call, X, pred):
            t0 = call * TB
            tb = min(TB, NT - t0)
            n = tb * M * P
            # zero the unselected half of each pair token
            nc.vector.copy_predicated(
                X[:, :n], pred[:, :n], zsel[:, 0:1].broadcast_to([P, n]))
            u_t = sb_u.tile([E, TB * P], BF16, tag="u")
            s_t = sb_u.tile([E, TB * P], F32, tag="sv")
            for grp in range(tb // 4):
                x4 = X[:, grp * 4 * M * P:(grp * 4 + 4) * M * P]
                x4 = x4.rearrange("p (t m c) -> p t m c", m=M, c=P)
                psY = ps_mm.tile([P, 4 * P], F32, tag="mm")
                psY4 = psY[:].rearrange("p (t c) -> p t c", c=P)
                for m in range(M):
                    nc.tensor.matmul(psY4[:], lhsT=wvc[:, m * P:(m + 1) * P],
                                     rhs=x4[:, :, m, :],
                                     start=(m == 0), stop=(m == M - 1))
                s = sb_ep.tile([E, 4 * P], F32, tag="s")
                nc.scalar.activation(out=s[:], in_=psY[0:E, :],
                                     func=AF.Sigmoid, bias=bvc[:], scale=1.0)
                lg = grp * 4 * P
                nc.vector.scalar_tensor_tensor(
                    out=u_t[:, lg:lg + 4 * P], in0=psY[E:2 * E, :],
                    scalar=bvc2[:], in1=s[:], op0=ALU.add, op1=ALU.add)
                sq = sb_ep.tile([E, 4 * P], F32, tag="sq")
                nc.vector.tensor_mul(sq[:], u_t[:, lg:lg + 4 * P],
                                     u_t[:, lg:lg + 4 * P])
                psS = ps_s.tile([E, 4 * P], F32, tag="ssum")
                nc.tensor.matmul(psS[:], lhsT=ones[:], rhs=sq[:],
                                 start=True, stop=True)
                nc.scalar.add(s_t[:, lg:lg + 4 * P], psS[:], epsv[:])

            nb = tb * P
            nc.vector.reciprocal_approx_fast(out=s_t[:, :nb], in_=s_t[:, :nb])
            nc.scalar.activation(out=s_t[:, :nb], in_=s_t[:, :nb],
                                 func=AF.Sqrt, bias=epsv[:], scale=1.0)
            nc.vector.tensor_mul(u_t[:, :nb], u_t[:, :nb], s_t[:, :nb])
            cvA = t_cembA[it][:].rearrange("(p t) e -> p t e", t=TA + 1)
            cvB = t_cembB[it][:].rearrange("(p t) e -> p t e", t=NTB + 1)
            for grp in range(tb // 4):
                g4 = t0 + grp * 4
                stage = sb_ep.tile([P, 4 * E], F32, tag="cstage")
                for tt in range(4):
                    lt = grp * 4 + tt
                    tp = ps_tp.tile([P, E], BF16, tag="tp")
                    nc.tensor.transpose(tp[:], u_t[:, lt * P:(lt + 1) * P],
                                        ident_bf[0:E, 0:E])
                    if tt % 2 == 0:
                        nc.scalar.copy(stage[:, tt * E:(tt + 1) * E], tp[:])
                    else:
                        nc.vector.tensor_copy(stage[:, tt * E:(tt + 1) * E],
                                              tp[:])
                st4 = stage[:].rearrange("p (t e) -> p t e", e=E)
                if g4 + 4 <= TA:
                    nc.sync.dma_start(cvA[:, g4:g4 + 4, :], st4)
                if g4 >= TB0:
                    nc.sync.dma_start(cvB[:, g4 - TB0:g4 - TB0 + 4, :], st4)

        def issue_x2_gather(it, call):
            g0 = call * GB
            gb = min(GB, NG - g0)
            n2 = gb * C * P
            src_t = t_cembA[it] if call < plan.X2_SPLIT else t_cembB[it]
            x2buf = sb_x2.tile([P, GB * C * E], F32, tag="x2buf")
            col0 = g0 * C * P // 16
            nc.gpsimd.dma_gather(
                x2buf[:, :gb * C * E].rearrange("p (r e) -> p r e", e=E),
                src_t[:], idxx2[:, col0:col0 + n2 // 16], n2, n2, E,
                single_packet=False)
            return x2buf

        def consume_x2_call(it, call, x2buf, cur):
            g0 = call * GB
            gb = min(GB, NG - g0)
            x2T = [sb_xt.tile([P, GB * P], BF16, tag=f"x2T{k}",
                              name=f"x2T{k}") for k in range(KC2)]
            for gg in range(gb):
                for k in range(KC2):
                    tp = ps_tp.tile([P, P], F32, tag="tp")
                    nc.tensor.transpose(
                        tp[:], x2buf[:, gg * C * E + k * P:
                                     gg * C * E + (k + 1) * P], ident[:])
                    if (gg * KC2 + k) % 2 == 0:
                        nc.scalar.copy(x2T[k][:, gg * P:(gg + 1) * P], tp[:])
                    else:
                        nc.vector.tensor_copy(x2T[k][:, gg * P:(gg + 1) * P],
                                              tp[:])
            psZ = ps_mm.tile([P, GB * P], F32, tag="mm")
            for k in range(KC2):
                nc.tensor.matmul(psZ[:, :gb * P], lhsT=wcc[:, k * P:(k + 1) * P],
                                 rhs=x2T[k][:, :gb * P],
                                 start=(k == 0), stop=(k == KC2 - 1))
            s2 = sb_ep.tile([E, GB * P], F32, tag="s")
            nc.scalar.activation(out=s2[:, :gb * P], in_=psZ[0:E, :gb * P],
                                 func=AF.Sigmoid, bias=bcc[:], scale=1.0)
            u2_t = sb_u.tile([E, GB * P], BF16, tag="u")
            s2_t = sb_u.tile([E, GB * P], F32, tag="sv")
            nc.vector.scalar_tensor_tensor(
                out=u2_t[:, :gb * P], in0=psZ[E:2 * E, :gb * P],
                scalar=bcc2[:], in1=s2[:, :gb * P], op0=ALU.add, op1=ALU.add)
            sq2 = sb_ep.tile([E, GB * P], F32, tag="sq")
            nc.vector.tensor_mul(sq2[:, :gb * P], u2_t[:, :gb * P],
                                 u2_t[:, :gb * P])
            psS2 = ps_s.tile([E, GB * P], F32, tag="ssum")
            nc.tensor.matmul(psS2[:, :gb * P], lhsT=ones[:],
                             rhs=sq2[:, :gb * P], start=True, stop=True)
            nc.scalar.add(s2_t[:, :gb * P], psS2[:, :gb * P], epsv[:])

            b0, b1 = g0 * P, (g0 + gb) * P
            dst = shardE[1] if it == 0 else shardE[0]
            nc.vector.reciprocal_approx_fast(out=s2_t[:, :gb * P],
                                             in_=s2_t[:, :gb * P])
            nc.scalar.activation(out=s2_t[:, :gb * P], in_=s2_t[:, :gb * P],
                                 func=AF.Sqrt, bias=epsv[:], scale=1.0)
            nc.vector.tensor_mul(dst[:, b0:b1], u2_t[:, :gb * P],
                                 s2_t[:, :gb * P])
            nc.vector.copy_predicated(dst[:, b0:b1], nocl[:, b0:b1],
                                      cur[:, b0:b1])
            if it == 0:
                nc.vector.tensor_copy(newvE_bf[:, b0:b1], dst[:, b0:b1])
                for gg in range(gb):
                    g = g0 + gg
                    psN = ps_s.tile([E, P], F32, tag="ssum")
                    nc.tensor.matmul(psN[:], lhsT=wneg[:],
                                     rhs=newvE_bf[:, g * P:(g + 1) * P],
                                     start=True, stop=True)
                    nc.scalar.add(negE[:, g * P:(g + 1) * P], psN[:], bneg[:])
                    tpa = ps_tp.tile([P, E], BF16, tag="tp")
                    nc.tensor.transpose(tpa[:], newvE_bf[:, g * P:(g + 1) * P],
                                        ident_bf[0:E, 0:E])
                    nc.scalar.copy(pairbuf[:, g * 2 * E:g * 2 * E + E], tpa[:])
                    tpb = ps_tp.tile([P, E], BF16, tag="tp")
                    nc.tensor.transpose(tpb[:], negE[:, g * P:(g + 1) * P],
                                        ident_bf[0:E, 0:E])
                    nc.vector.tensor_copy(
                        pairbuf[:, g * 2 * E + E:(g + 1) * 2 * E], tpb[:])

        for it in range(n_iters):
            ctab = t_gtab1 if it == 0 else t_gtab2f
            cur = shardE[it]

            # ---- clause stage (x2 A-gathers injected mid-phase) ----
            x2bufs = {}
            for call in range(ncalls):
                X, pred = issue_clause_gather(it, call, prepare=False)
                consume_clause_call(it, call, X, pred)
                if call == ncalls - 3 and stages >= 2:
                    for c in range(plan.X2_SPLIT):
                        x2bufs[c] = issue_x2_gather(it, c)

            if stages < 2:
                break

            # ---- variable stage ----
            for c in range(plan.X2_SPLIT, ncalls2):
                x2bufs[c] = issue_x2_gather(it, c)
            for call in range(ncalls2):
                consume_x2_call(it, call, x2bufs[call], cur)

            if it == 0:
                nc.sync.dma_start(
                    t_ag_in[:].rearrange("(g p) c -> p g c", p=P),
                    pairbuf[:].rearrange("p (g c) -> p g c", c=2 * E))
                if stages >= 4:
                    nc.gpsimd.collective_compute(
                        "AllGather", ALU.bypass,
                        replica_groups=[list(range(NCORES))],
                        ins=[t_ag_in[:]], outs=[t_gtab2[0:plan.GT_DATA, :]])
                    nc.sync.dma_start(t_gtab2f[0:plan.GT_DATA, :],
                                      t_gtab2[0:plan.GT_DATA, :])
            else:
                nc.sync.dma_start(t_out[:], shardE[0][:])

        if n_iters == 1 or stages < 2:
            nc.sync.dma_start(t_out[:], shardE[0][:])
    nc.compile()
    return nc


def _prep_inputs(plan: HostPlan, inputs):
    V, G, C, M, E = plan.V, plan.G, plan.C, plan.M, plan.E
    VS, NG, VSP = plan.VS, plan.NG, plan.VSP
    init = np.concatenate([
        _l2norm(np.asarray(inputs["emb_table"], np.float32)),
        np.tile(_l2norm(np.asarray(inputs["tseitin_emb"], np.float32))[None, :],
                (V - G, 1))], axis=0)
    W_neg = np.asarray(inputs["W_neg"], np.float32)
    b_neg = np.asarray(inputs["b_neg"], np.float32)
    neg0 = init @ W_neg + b_neg
    false_emb = np.asarray(inputs["false_emb"], np.float32)

    gtab1 = np.zeros((plan.GT_ROWS, 2 * E), np.float32)
    gtab1[plan.gtab_row] = np.concatenate([init, neg0], axis=1)
    gtab1[plan.FALSE_TOK] = np.concatenate([false_emb, false_emb])

    wvc_full = np.concatenate([np.asarray(inputs["vc_W1"], np.float32),
                               np.asarray(inputs["vc_W2"], np.float32)], 1)
    wcc_full = np.concatenate([np.asarray(inputs["cc_W1"], np.float32),
                               np.asarray(inputs["cc_W2"], np.float32)], 1)
    wvc_sb = np.concatenate(
        [np.vstack([wvc_full[m * E:(m + 1) * E]] * 2) for m in range(M)], 1)
    wcc_sb = np.concatenate([wcc_full[k * P:(k + 1) * P]
                             for k in range((C * E) // P)], 1)

    import ml_dtypes
    bf = lambda x: np.asarray(x, ml_dtypes.bfloat16)

    in_maps = []
    for k in range(NCORES):
        vlo = k * VS
        shE = np.zeros((E, VSP), np.float32)
        shE[:, :VS] = init[vlo:vlo + VS].T
        in_maps.append({
            "gtab1": bf(gtab1),
            "idx_main": plan.idx_main[k],
            "pred_neg": plan.pred_neg[k],
            "idx_x2": plan.idx_x2[k],
            "shard0E": shE,
            "noclE": plan.noclE[k],
            "wvc": bf(wvc_sb), "wcc": bf(wcc_sb),
            "wneg": bf(W_neg),
            "bvc": np.asarray(inputs["vc_b1"], np.float32)[:, None],
            "bvc2": np.asarray(inputs["vc_b2"], np.float32)[:, None],
            "bcc": np.asarray(inputs["cc_b1"], np.float32)[:, None],
            "bcc2": np.asarray(inputs["cc_b2"], np.float32)[:, None],
            "bneg": b_neg[:, None],
            "false_pair": bf(np.concatenate([false_emb, false_emb])[None, :]),
        })
    return in_maps


def run(inputs, V, G, C, M, E, trace=False, stages=4):
    plan = HostPlan(V, G, C, M, E,
                    np.asarray(inputs["clause_lits"]),
                    np.asarray(inputs["lit_neg"]),
                    np.asarray(inputs["lit_valid"]),
                    np.asarray(inputs["clause_valid"]))
    nc = build_program(plan, stages=stages)
    in_maps = _prep_inputs(plan, inputs)
    res = run_bass_kernel_spmd(nc, in_maps, core_ids=list(range(NCORES)),
                               trace=trace)
    VS = plan.VS
    out = np.empty((V, E), np.float32)
    for k in range(NCORES):
        raw = res.results[k]["out_shardE"]           # [64, VSP]
        out[k * VS:(k + 1) * VS] = raw.T[:VS]
    return out, res


def kernel(**inputs) -> np.ndarray:
    out, _ = run(inputs, V=20000, G=16000, C=8, M=6, E=64)
    return out



# revision 5
# speedup vs baseline: 1.2315x; 1.2315x over previous
"""Trainium2 Bass kernel for nn_Encoder (GNN message passing / SAT encoder).

Design v2 (8 NeuronCores, data-parallel over V):
  - Global pair-token table gtab[row(v)] = (emb_v bf16[64] || neg_emb_v bf16[64])
    (256B rows, 20481 rows < int16 range). Iter-1 table host-built; iter-2
    table produced by ONE AllGather of per-shard pair rows. No per-core
    compact-table rebuild.
  - Clause stage: transpose-mode dma_gather (16-bit xbar) returns literal
    tokens E-MAJOR: partition p = bf16 element p -> partitions 0:64 plain,
    64:128 negated. A single copy_predicated per call selects the negated
    half per literal. No PE transposes or PSUM->SBUF shuffles; matmuls are
    64-contraction bf16 with batched 4-tile moving dim.
  - l2norm is deferred: sum-of-squares via a ones-matmul (partition
    broadcast), then ONE Rsqrt activation per stage (no act-table thrash),
    then one big elementwise multiply.
  - Variable stage: row gather of c_emb (fp32, 256B rows) from DRAM into
    dense (v,c) layout, PE transposes, 128-contraction bf16 matmuls.
"""

import math
from contextlib import ExitStack

import numpy as np

import concourse.bass as bass
import concourse.bacc as bacc
import concourse.mybir as mybir
import concourse.tile as tile
from concourse.bass_utils import run_bass_kernel_spmd
from concourse.masks import make_identity

F32 = mybir.dt.float32
BF16 = mybir.dt.bfloat16
I16 = mybir.dt.int16
I8 = mybir.dt.int8
AF = mybir.ActivationFunctionType
ALU = mybir.AluOpType

NCORES = 8
P = 128


def _l2norm(x):
    n = np.sqrt(np.sum(x * x, axis=-1, keepdims=True))
    return x / np.maximum(n, 1e-12)


def _wrap_idx(flat):
    """dma_gather idx layout: idx j -> [j%16, j//16] int16, replicated x8."""
    n = len(flat)
    assert n % 16 == 0
    w = np.asarray(flat, np.int16).reshape(-1, 16).T.copy()
    return np.tile(w, (8, 1))


def _ceil_to(x, m):
    return ((x + m - 1) // m) * m


class HostPlan:
    """Static index math; identical program across cores (SPMD)."""

    def __init__(self, V, G, C, M, E, clause_lits, lit_neg, lit_valid,
                 clause_valid):
        self.V, self.G, self.C, self.M, self.E = V, G, C, M, E
        VS = V // NCORES
        self.VS = VS
        self.NG = _ceil_to(VS, P) // P
        self.VSP = self.NG * P

        # literal rotation (prepare_variables)
        var_ids = np.arange(V)[:, None, None]
        ind = np.argmax((clause_lits == var_ids) & (lit_valid > 0), axis=-1)
        p = np.arange(M)[None, None, :]
        ind_b = ind[:, :, None]
        gidx = np.where(p == 0, ind_b, np.where(p - 1 < ind_b, p - 1, p))
        self.psrc = np.take_along_axis(clause_lits, gidx, 2)   # [V,C,M]
        self.pneg = np.take_along_axis(lit_neg, gidx, 2) > 0
        self.pval = np.take_along_axis(lit_valid, gidx, 2) > 0
        self.cval = clause_valid > 0                           # [V,C]
        self.has_clause = self.cval.any(1)                     # [V]

        # global pair-table row of var v
        vv = np.arange(V)
        self.gtab_row = (vv // VS) * self.VSP + (vv % VS)
        self.GT_DATA = NCORES * self.VSP            # 20480
        self.FALSE_TOK = self.GT_DATA               # row 20480
        self.GT_ROWS = self.GT_DATA + 1
        assert self.GT_ROWS < 32767

        # per-core valid (v,c) rows, v-major
        self.rows = []
        for k in range(NCORES):
            vlo = k * VS
            rv, rc = np.nonzero(self.cval[vlo:vlo + VS])
            self.rows.append((len(rv), rv, rc))
        maxrows = max(r[0] for r in self.rows)
        self.NTILES = _ceil_to(_ceil_to(maxrows, P) // P, 4)
        self.NR = self.NTILES * P
        # split c_emb into two overlapping tile-range tensors so early x2
        # gathers depend only on the first half of the clause stage.
        # Bounds are data-driven: A must cover rows of x2 calls < X2_SPLIT,
        # B must cover rows of the remaining calls.
        self.X2_SPLIT = 2
        split_var = self.X2_SPLIT * 4 * P
        rmax_A = max(int(np.count_nonzero(r[1] < split_var))
                     for r in self.rows)
        rmin_B = min(int(np.searchsorted(r[1], split_var))
                     for r in self.rows)
        self.TA = min(_ceil_to(_ceil_to(rmax_A, P) // P + 1, 4), self.NTILES)
        self.TB0 = max((rmin_B // P - 1) // 4 * 4, 0)
        self.NTB = self.NTILES - self.TB0
        self.ZROW_A = self.TA
        self.ZROW_B = self.NTB
        self.CEMB_A_ROWS = P * (self.TA + 1)
        self.CEMB_B_ROWS = P * (self.NTB + 1)

        # per-core tensors
        self.idx_main = []   # [128, NR*M/16] i16  (stream (t, m, p))
        self.pred_neg = []   # [64, NR*M] int8
        self.idx_x2 = []     # [128, VSP*C/16] i16 (stream (g, c, p))
        self.noclE = []      # [64, VSP] int8 (1 = keep old value)
        for k in range(NCORES):
            vlo = k * VS
            nr, rv, rc = self.rows[k]
            src = self.psrc[vlo + rv, rc]          # [nr, M]
            neg = self.pneg[vlo + rv, rc]
            val = self.pval[vlo + rv, rc]
            tok = np.where(val, self.gtab_row[src], self.FALSE_TOK)
            full = np.full((self.NR, M), self.FALSE_TOK, np.int64)
            full[:nr] = tok
            pr = np.zeros((self.NR, M), np.int8)
            pr[:nr] = (val & neg)
            # stream order (t, m, p)
            st = full.reshape(self.NTILES, P, M).transpose(0, 2, 1).reshape(-1)
            sp = pr.reshape(self.NTILES, P, M).transpose(0, 2, 1).reshape(-1)
            self.idx_main.append(_wrap_idx(st))
            pred = np.concatenate([np.tile(sp[None, :], (64, 1)),
                                   np.tile(1 - sp[None, :], (64, 1))], axis=0)
            self.pred_neg.append(pred.copy())

            # x2: dense (v, c) -> cemb dram row, A/B-relative per call
            r = np.arange(nr)
            rowA = np.full((self.VSP, C), self.ZROW_A, np.int64)
            rowA[rv, rc] = (r % P) * (self.TA + 1) + r // P
            rowB = np.full((self.VSP, C), self.ZROW_B, np.int64)
            rowB[rv, rc] = (r % P) * (self.NTB + 1) + (r // P - self.TB0)
            split_var = self.X2_SPLIT * 4 * P
            assert np.count_nonzero(rv < split_var) <= self.TA * P
            assert np.searchsorted(rv, split_var) >= self.TB0 * P
            xa = rowA.reshape(self.NG, P, C).transpose(0, 2, 1)
            xb = rowB.reshape(self.NG, P, C).transpose(0, 2, 1)
            ns = self.X2_SPLIT * 4
            x2 = np.concatenate([xa[:ns].reshape(-1), xb[ns:].reshape(-1)])
            self.idx_x2.append(_wrap_idx(x2))

            hc = np.ones((self.VSP,), np.int8)
            hc[:VS] = (~self.has_clause[vlo:vlo + VS]).astype(np.int8)
            self.noclE.append(np.tile(hc[None, :], (64, 1)).copy())


def build_program(plan: HostPlan, TB=8, GB=4, stages=99):
    V, C, M, E = plan.V, plan.C, plan.M, plan.E
    NT, NG, VSP = plan.NTILES, plan.NG, plan.VSP
    NR = plan.NR
    nc = bacc.Bacc("TRN2", num_devices=NCORES,
                   dynamic_dma_scratch_size=16384, num_swdge_queues=4)

    ein = lambda name, shape, dt=F32: nc.dram_tensor(
        name, shape, dt, kind="ExternalInput")
    t_gtab1 = ein("gtab1", [plan.GT_ROWS, 2 * E], BF16)
    t_idx_main = ein("idx_main", [P, NR * M // 16], I16)
    t_pred = ein("pred_neg", [P, NR * M], I8)
    t_idx_x2 = ein("idx_x2", [P, VSP * C // 16], I16)
    t_shard0 = ein("shard0E", [E, VSP])
    t_nocl = ein("noclE", [E, VSP], I8)
    t_wvc = ein("wvc", [P, M * P], BF16)
    t_wcc = ein("wcc", [P, (C // 2) * P], BF16)
    t_wneg = ein("wneg", [E, E], BF16)
    t_bvc = ein("bvc", [E, 1])
    t_bvc2 = ein("bvc2", [E, 1])
    t_bcc = ein("bcc", [E, 1])
    t_bcc2 = ein("bcc2", [E, 1])
    t_bneg = ein("bneg", [E, 1])
    t_false = ein("false_pair", [1, 2 * E], BF16)

    t_out = nc.dram_tensor("out_shardE", [E, VSP], F32, kind="ExternalOutput")

    t_gtab2 = nc.dram_tensor("gtab2", [plan.GT_ROWS, 2 * E], BF16,
                             addr_space="Shared")
    t_gtab2f = nc.dram_tensor("gtab2f", [plan.GT_ROWS, 2 * E], BF16,
                              kind="ExternalOutput")
    TA, TB0, NTB = plan.TA, plan.TB0, plan.NTB
    t_cembA = [nc.dram_tensor(f"cembA{i}", [plan.CEMB_A_ROWS, E], F32,
                              kind="ExternalOutput") for i in range(2)]
    t_cembB = [nc.dram_tensor(f"cembB{i}", [plan.CEMB_B_ROWS, E], F32,
                              kind="ExternalOutput") for i in range(2)]
    t_ag_in = nc.dram_tensor("ag_in", [VSP, 2 * E], BF16)
    t_warm_in = nc.dram_tensor("warm_in", [1, 2 * E], BF16)
    t_warm_out = nc.dram_tensor("warm_out", [NCORES, 2 * E], BF16)

    KC2 = (C * E) // P  # 4 contraction chunks for variable stage

    with tile.TileContext(nc) as tc, ExitStack() as ctx:
        const = ctx.enter_context(tc.tile_pool(name="const", bufs=1))
        sb_x = ctx.enter_context(tc.tile_pool(name="sb_x", bufs=3))
        sb_pred = ctx.enter_context(tc.tile_pool(name="sb_pred", bufs=2))
        sb_x2 = ctx.enter_context(tc.tile_pool(name="sb_x2", bufs=5))
        sb_ep = ctx.enter_context(tc.tile_pool(name="sb_ep", bufs=3))
        sb_u = ctx.enter_context(tc.tile_pool(name="sb_u", bufs=3))
        sb_xt = ctx.enter_context(tc.tile_pool(name="sb_xt", bufs=2))
        ps_mm = ctx.enter_context(tc.tile_pool(name="ps_mm", bufs=2, space="PSUM"))
        ps_s = ctx.enter_context(tc.tile_pool(name="ps_s", bufs=2, space="PSUM"))
        ps_tp = ctx.enter_context(tc.tile_pool(name="ps_tp", bufs=2, space="PSUM"))

        # ---- constants ----
        ident = const.tile([P, P], F32)
        make_identity(nc, ident)
        ident_bf = const.tile([P, P], BF16)
        nc.vector.tensor_copy(ident_bf[:], ident[:])
        ones = const.tile([E, E], F32)
        nc.vector.memset(ones[:], 1.0)
        wvc = const.tile([P, M * P], BF16)
        nc.sync.dma_start(wvc[:], t_wvc[:])
        wcc = const.tile([P, KC2 * P], BF16)
        nc.sync.dma_start(wcc[:], t_wcc[:])
        wneg = const.tile([E, E], BF16)
        nc.sync.dma_start(wneg[:], t_wneg[:])
        bvc = const.tile([E, 1], F32)
        nc.sync.dma_start(bvc[:], t_bvc[:])
        bvc2 = const.tile([E, 1], F32)
        nc.sync.dma_start(bvc2[:], t_bvc2[:])
        bcc = const.tile([E, 1], F32)
        nc.sync.dma_start(bcc[:], t_bcc[:])
        bcc2 = const.tile([E, 1], F32)
        nc.sync.dma_start(bcc2[:], t_bcc2[:])
        bneg = const.tile([E, 1], F32)
        nc.sync.dma_start(bneg[:], t_bneg[:])
        idxm = const.tile([P, NR * M // 16], I16)
        nc.sync.dma_start(idxm[:], t_idx_main[:])
        idxx2 = const.tile([P, VSP * C // 16], I16)
        nc.sync.dma_start(idxx2[:], t_idx_x2[:])
        nocl = const.tile([E, VSP], I8)
        nc.sync.dma_start(nocl[:], t_nocl[:])
        false_sb = const.tile([1, 2 * E], BF16)
        nc.sync.dma_start(false_sb[:], t_false[:])
        zero64 = const.tile([P, E], F32)
        nc.vector.memset(zero64[:], 0.0)
        epsv = const.tile([E, 1], F32)
        nc.vector.memset(epsv[:], 1e-24)
        zsel = const.tile([P, 1], BF16)
        nc.vector.memset(zsel[:], 0.0)

        shardE = [const.tile([E, VSP], F32, tag=f"shE{i}", name=f"shE{i}")
                  for i in range(2)]
        nc.sync.dma_start(shardE[0][:], t_shard0[:])

        # startup: zero rows of cemb tensors + false row of gtab2
        for i in range(2):
            cva = t_cembA[i][:].rearrange("(p t) e -> p t e", t=TA + 1)
            nc.sync.dma_start(cva[:, TA:TA + 1, :],
                              zero64[:].rearrange("p (o e) -> p o e", o=1))
            cvb = t_cembB[i][:].rearrange("(p t) e -> p t e", t=NTB + 1)
            nc.sync.dma_start(cvb[:, NTB:NTB + 1, :],
                              zero64[:].rearrange("p (o e) -> p o e", o=1))
        nc.sync.dma_start(t_gtab2f[plan.FALSE_TOK:plan.FALSE_TOK + 1, :],
                          false_sb[:])
        nc.sync.dma_start(t_warm_in[:], false_sb[:])
        nc.gpsimd.collective_compute(
            "AllGather", ALU.bypass, replica_groups=[list(range(NCORES))],
            ins=[t_warm_in[:]], outs=[t_warm_out[:]])

        # persistent big buffers
        newvE_bf = const.tile([E, VSP], BF16)
        negE = const.tile([E, VSP], BF16)
        pairbuf = const.tile([P, NG * 2 * E], BF16)

        n_iters = 1 if stages < 3 else 2
        ncalls = math.ceil(NT / TB)
        ncalls2 = math.ceil(NG / GB)
        NPRE = 3 if stages >= 5 else 0
        sem_ctr = [0]

        def new_sem():
            sem_ctr[0] += 1
            return nc.alloc_semaphore(f"swdge{sem_ctr[0]}")

        def issue_clause_gather(it, call, prepare):
            ctab = t_gtab1 if it == 0 else t_gtab2f
            t0 = call * TB
            tb = min(TB, NT - t0)
            n = tb * M * P
            col0 = t0 * M * P
            X = sb_x.tile([P, TB * M * P], BF16, tag="X")
            kw = {}
            if prepare:
                kw = dict(prepare_only=True, sem=new_sem())
            nc.gpsimd.dma_gather(
                X[:, :n].rearrange("p (o n) -> p o n", o=1),
                ctab[:], idxm[:, col0 // 16:(col0 + n) // 16], n, n,
                2 * E, transpose=True, single_packet=False,
                queue_num=0, **kw)
            pred = sb_pred.tile([P, TB * M * P], I8, tag="pred")
            nc.sync.dma_start(pred[:, :n], t_pred[:, col0:col0 + n])
            return X, pred

        def consume_clause_call(it, call, X, pred):
            t0 = call * TB
            tb = min(TB, NT - t0)
            n = tb * M * P
            # zero the unselected half of each pair token
            nc.vector.copy_predicated(
                X[:, :n], pred[:, :n], zsel[:, 0:1].broadcast_to([P, n]))
            u_t = sb_u.tile([E, TB * P], BF16, tag="u")
            s_t = sb_u.tile([E, TB * P], F32, tag="sv")
            for grp in range(tb // 4):
                x4 = X[:, grp * 4 * M * P:(grp * 4 + 4) * M * P]
                x4 = x4.rearrange("p (t m c) -> p t m c", m=M, c=P)
                psY = ps_mm.tile([P, 4 * P], F32, tag="mm")
                psY4 = psY[:].rearrange("p (t c) -> p t c", c=P)
                for m in range(M):
                    nc.tensor.matmul(psY4[:], lhsT=wvc[:, m * P:(m + 1) * P],
                                     rhs=x4[:, :, m, :],
                                     start=(m == 0), stop=(m == M - 1))
                s = sb_ep.tile([E, 4 * P], F32, tag="s")
                nc.scalar.activation(out=s[:], in_=psY[0:E, :],
                                     func=AF.Sigmoid, bias=bvc[:], scale=1.0)
                lg = grp * 4 * P
                nc.vector.scalar_tensor_tensor(
                    out=u_t[:, lg:lg + 4 * P], in0=psY[E:2 * E, :],
                    scalar=bvc2[:], in1=s[:], op0=ALU.add, op1=ALU.add)
                sq = sb_ep.tile([E, 4 * P], F32, tag="sq")
                nc.vector.tensor_mul(sq[:], u_t[:, lg:lg + 4 * P],
                                     u_t[:, lg:lg + 4 * P])
                psS = ps_s.tile([E, 4 * P], F32, tag="ssum")
                nc.tensor.matmul(psS[:], lhsT=ones[:], rhs=sq[:],
                                 start=True, stop=True)
                nc.scalar.add(s_t[:, lg:lg + 4 * P], psS[:], epsv[:])

            nb = tb * P
            nc.vector.reciprocal_approx_fast(out=s_t[:, :nb], in_=s_t[:, :nb])
            nc.scalar.activation(out=s_t[:, :nb], in_=s_t[:, :nb],
                                 func=AF.Sqrt, bias=epsv[:], scale=1.0)
            nc.vector.tensor_mul(u_t[:, :nb], u_t[:, :nb], s_t[:, :nb])
            cvA = t_cembA[it][:].rearrange("(p t) e -> p t e", t=TA + 1)
            cvB = t_cembB[it][:].rearrange("(p t) e -> p t e", t=NTB + 1)
            for grp in range(tb // 4):
                g4 = t0 + grp * 4
                stage = sb_ep.tile([P, 4 * E], F32, tag="cstage")
                for tt in range(4):
                    lt = grp * 4 + tt
                    tp = ps_tp.tile([P, E], BF16, tag="tp")
                    nc.tensor.transpose(tp[:], u_t[:, lt * P:(lt + 1) * P],
                                        ident_bf[0:E, 0:E])
                    if tt % 2 == 0:
                        nc.scalar.copy(stage[:, tt * E:(tt + 1) * E], tp[:])
                    else:
                        nc.vector.tensor_copy(stage[:, tt * E:(tt + 1) * E],
                                              tp[:])
                st4 = stage[:].rearrange("p (t e) -> p t e", e=E)
                if g4 + 4 <= TA:
                    nc.sync.dma_start(cvA[:, g4:g4 + 4, :], st4)
                if g4 >= TB0:
                    nc.sync.dma_start(cvB[:, g4 - TB0:g4 - TB0 + 4, :], st4)

        def issue_x2_gather(it, call):
            g0 = call * GB
            gb = min(GB, NG - g0)
            n2 = gb * C * P
            src_t = t_cembA[it] if call < plan.X2_SPLIT else t_cembB[it]
            x2buf = sb_x2.tile([P, GB * C * E], F32, tag="x2buf")
            col0 = g0 * C * P // 16
            nc.gpsimd.dma_gather(
                x2buf[:, :gb * C * E].rearrange("p (r e) -> p r e", e=E),
                src_t[:], idxx2[:, col0:col0 + n2 // 16], n2, n2, E,
                single_packet=False, queue_num=0)
            return x2buf

        def consume_x2_call(it, call, x2buf, cur):
            g0 = call * GB
            gb = min(GB, NG - g0)
            x2T = [sb_xt.tile([P, GB * P], BF16, tag=f"x2T{k}",
                              name=f"x2T{k}") for k in range(KC2)]
            for gg in range(gb):
                for k in range(KC2):
                    tp = ps_tp.tile([P, P], F32, tag="tp")
                    nc.tensor.transpose(
                        tp[:], x2buf[:, gg * C * E + k * P:
                                     gg * C * E + (k + 1) * P], ident[:])
                    if (gg * KC2 + k) % 2 == 0:
                        nc.scalar.copy(x2T[k][:, gg * P:(gg + 1) * P], tp[:])
                    else:
                        nc.vector.tensor_copy(x2T[k][:, gg * P:(gg + 1) * P],
                                              tp[:])
            psZ = ps_mm.tile([P, GB * P], F32, tag="mm")
            for k in range(KC2):
                nc.tensor.matmul(psZ[:, :gb * P], lhsT=wcc[:, k * P:(k + 1) * P],
                                 rhs=x2T[k][:, :gb * P],
                                 start=(k == 0), stop=(k == KC2 - 1))
            s2 = sb_ep.tile([E, GB * P], F32, tag="s")
            nc.scalar.activation(out=s2[:, :gb * P], in_=psZ[0:E, :gb * P],
                                 func=AF.Sigmoid, bias=bcc[:], scale=1.0)
            u2_t = sb_u.tile([E, GB * P], BF16, tag="u")
            s2_t = sb_u.tile([E, GB * P], F32, tag="sv")
            nc.vector.scalar_tensor_tensor(
                out=u2_t[:, :gb * P], in0=psZ[E:2 * E, :gb * P],
                scalar=bcc2[:], in1=s2[:, :gb * P], op0=ALU.add, op1=ALU.add)
            sq2 = sb_ep.tile([E, GB * P], F32, tag="sq")
            nc.vector.tensor_mul(sq2[:, :gb * P], u2_t[:, :gb * P],
                                 u2_t[:, :gb * P])
            psS2 = ps_s.tile([E, GB * P], F32, tag="ssum")
            nc.tensor.matmul(psS2[:, :gb * P], lhsT=ones[:],
                             rhs=sq2[:, :gb * P], start=True, stop=True)
            nc.scalar.add(s2_t[:, :gb * P], psS2[:, :gb * P], epsv[:])

            b0, b1 = g0 * P, (g0 + gb) * P
            dst = shardE[1] if it == 0 else shardE[0]
            nc.vector.reciprocal_approx_fast(out=s2_t[:, :gb * P],
                                             in_=s2_t[:, :gb * P])
            nc.scalar.activation(out=s2_t[:, :gb * P], in_=s2_t[:, :gb * P],
                                 func=AF.Sqrt, bias=epsv[:], scale=1.0)
            nc.vector.tensor_mul(dst[:, b0:b1], u2_t[:, :gb * P],
                                 s2_t[:, :gb * P])
            nc.vector.copy_predicated(dst[:, b0:b1], nocl[:, b0:b1],
                                      cur[:, b0:b1])
            if it == 0:
                nc.vector.tensor_copy(newvE_bf[:, b0:b1], dst[:, b0:b1])
                for gg in range(gb):
                    g = g0 + gg
                    psN = ps_s.tile([E, P], F32, tag="ssum")
                    nc.tensor.matmul(psN[:], lhsT=wneg[:],
                                     rhs=newvE_bf[:, g * P:(g + 1) * P],
                                     start=True, stop=True)
                    nc.scalar.add(negE[:, g * P:(g + 1) * P], psN[:], bneg[:])
                    tpa = ps_tp.tile([P, E], BF16, tag="tp")
                    nc.tensor.transpose(tpa[:], newvE_bf[:, g * P:(g + 1) * P],
                                        ident_bf[0:E, 0:E])
                    nc.scalar.copy(pairbuf[:, g * 2 * E:g * 2 * E + E], tpa[:])
                    tpb = ps_tp.tile([P, E], BF16, tag="tp")
                    nc.tensor.transpose(tpb[:], negE[:, g * P:(g + 1) * P],
                                        ident_bf[0:E, 0:E])
                    nc.vector.tensor_copy(
                        pairbuf[:, g * 2 * E + E:(g + 1) * 2 * E], tpb[:])

        for it in range(n_iters):
            ctab = t_gtab1 if it == 0 else t_gtab2f
            cur = shardE[it]

            # ---- clause stage (x2 A-gathers injected mid-phase) ----
            x2bufs = {}
            for call in range(ncalls):
                X, pred = issue_clause_gather(it, call, prepare=False)
                consume_clause_call(it, call, X, pred)
                if call == ncalls - 3 and stages >= 2:
                    for c in range(plan.X2_SPLIT):
                        x2bufs[c] = issue_x2_gather(it, c)

            if stages < 2:
                break

            # ---- variable stage ----
            for c in range(plan.X2_SPLIT, ncalls2):
                x2bufs[c] = issue_x2_gather(it, c)
            for call in range(ncalls2):
                consume_x2_call(it, call, x2bufs[call], cur)

            if it == 0:
                nc.sync.dma_start(
                    t_ag_in[:].rearrange("(g p) c -> p g c", p=P),
                    pairbuf[:].rearrange("p (g c) -> p g c", c=2 * E))
                if stages >= 4:
                    nc.gpsimd.collective_compute(
                        "AllGather", ALU.bypass,
                        replica_groups=[list(range(NCORES))],
                        ins=[t_ag_in[:]], outs=[t_gtab2[0:plan.GT_DATA, :]])
                    nc.sync.dma_start(t_gtab2f[0:plan.GT_DATA, :],
                                      t_gtab2[0:plan.GT_DATA, :])
            else:
                nc.sync.dma_start(t_out[:], shardE[0][:])

        if n_iters == 1 or stages < 2:
            nc.sync.dma_start(t_out[:], shardE[0][:])
    nc.compile()
    return nc


def _prep_inputs(plan: HostPlan, inputs):
    V, G, C, M, E = plan.V, plan.G, plan.C, plan.M, plan.E
    VS, NG, VSP = plan.VS, plan.NG, plan.VSP
    init = np.concatenate([
        _l2norm(np.asarray(inputs["emb_table"], np.float32)),
        np.tile(_l2norm(np.asarray(inputs["tseitin_emb"], np.float32))[None, :],
                (V - G, 1))], axis=0)
    W_neg = np.asarray(inputs["W_neg"], np.float32)
    b_neg = np.asarray(inputs["b_neg"], np.float32)
    neg0 = init @ W_neg + b_neg
    false_emb = np.asarray(inputs["false_emb"], np.float32)

    gtab1 = np.zeros((plan.GT_ROWS, 2 * E), np.float32)
    gtab1[plan.gtab_row] = np.concatenate([init, neg0], axis=1)
    gtab1[plan.FALSE_TOK] = np.concatenate([false_emb, false_emb])

    wvc_full = np.concatenate([np.asarray(inputs["vc_W1"], np.float32),
                               np.asarray(inputs["vc_W2"], np.float32)], 1)
    wcc_full = np.concatenate([np.asarray(inputs["cc_W1"], np.float32),
                               np.asarray(inputs["cc_W2"], np.float32)], 1)
    wvc_sb = np.concatenate(
        [np.vstack([wvc_full[m * E:(m + 1) * E]] * 2) for m in range(M)], 1)
    wcc_sb = np.concatenate([wcc_full[k * P:(k + 1) * P]
                             for k in range((C * E) // P)], 1)

    import ml_dtypes
    bf = lambda x: np.asarray(x, ml_dtypes.bfloat16)

    in_maps = []
    for k in range(NCORES):
        vlo = k * VS
        shE = np.zeros((E, VSP), np.float32)
        shE[:, :VS] = init[vlo:vlo + VS].T
        in_maps.append({
            "gtab1": bf(gtab1),
            "idx_main": plan.idx_main[k],
            "pred_neg": plan.pred_neg[k],
            "idx_x2": plan.idx_x2[k],
            "shard0E": shE,
            "noclE": plan.noclE[k],
            "wvc": bf(wvc_sb), "wcc": bf(wcc_sb),
            "wneg": bf(W_neg),
            "bvc": np.asarray(inputs["vc_b1"], np.float32)[:, None],
            "bvc2": np.asarray(inputs["vc_b2"], np.float32)[:, None],
            "bcc": np.asarray(inputs["cc_b1"], np.float32)[:, None],
            "bcc2": np.asarray(inputs["cc_b2"], np.float32)[:, None],
            "bneg": b_neg[:, None],
            "false_pair": bf(np.concatenate([false_emb, false_emb])[None, :]),
        })
    return in_maps


def run(inputs, V, G, C, M, E, trace=False, stages=4):
    plan = HostPlan(V, G, C, M, E,
                    np.asarray(inputs["clause_lits"]),
                    np.asarray(inputs["lit_neg"]),
                    np.asarray(inputs["lit_valid"]),
                    np.asarray(inputs["clause_valid"]))
    nc = build_program(plan, stages=stages)
    in_maps = _prep_inputs(plan, inputs)
    res = run_bass_kernel_spmd(nc, in_maps, core_ids=list(range(NCORES)),
                               trace=trace)
    VS = plan.VS
    out = np.empty((V, E), np.float32)
    for k in range(NCORES):
        raw = res.results[k]["out_shardE"]           # [64, VSP]
        out[k * VS:(k + 1) * VS] = raw.T[:VS]
    return out, res


def kernel(**inputs) -> np.ndarray:
    out, _ = run(inputs, V=20000, G=16000, C=8, M=6, E=64)
    return out

